# revision 1
# baseline (speedup 1.0000x reference)
"""CrossViewSwapAttention Trainium2 kernel.

Sharding: the 64 window groups L=X*Y are split across 8 cores (8 groups per
core); each window group's attention is fully local to a core. Host folds the
LayerNorm gains/biases, attn_scale, dh**-0.5, the mean-over-views 1/n and bp
into the weights, rearranges q/k/v to [L, tokens, d] token-major shards, and
re-assembles the output.
"""
import numpy as np

HEADS, DH, D = 4, 32, 128
LN_EPS = 1e-5
NCORES = 8
LPC = 8          # L window-groups per core
TQ, TK = 384, 480
QC, QCS = 3, 128  # q chunks
KC, KCS = 4, 120  # kv chunks

_prog = {}


def _build_program(debug=False):
    import concourse.bass as bass
    import concourse.tile as tile
    from concourse import mybir
    from concourse.tile import ScopedClock

    # -- walrus workaround: split tail-drain sem waits into single-wait NOPs --
    def _drain_and_barrier(self, tick_clock, wait_clock):
        nc = self.nc
        nop0 = nc.sync.nop()
        wait_clock.add_sem_waits(nop0.ins, ScopedClock({None: tick_clock.global_clock}))
        si = nop0.ins.sync_info
        waits = list(si.on_wait) if si is not None else []
        if len(waits) > 1:
            nop0.ins.sync_info = mybir.SyncInfo(on_wait=waits[:1], on_update=list(si.on_update))
            for w in waits[1:]:
                n = nc.sync.nop()
                n.ins.sync_info = mybir.SyncInfo(on_wait=[w], on_update=[])
        nc.sync.drain()
        nc.all_engine_barrier()
        assert self.sems is not None
        popped = nc._tile_sem_poison_stack.pop()
        assert popped is self._sem_poison
        nc.clear_and_free_semaphores(list(self.sems.allocated().values()))
        nc.all_engine_barrier()

    tile.TileContext._drain_and_barrier = _drain_and_barrier

    f32 = mybir.dt.float32
    nc = bass.Bass()
    d_q = nc.declare_dram_parameter("q", [LPC, TQ, D], f32, isOutput=False)
    d_k = nc.declare_dram_parameter("k", [LPC, TK, D], f32, isOutput=False)
    d_v = nc.declare_dram_parameter("v", [LPC, TK, D], f32, isOutput=False)
    d_skip = nc.declare_dram_parameter("skip", [LPC, 64, D], f32, isOutput=False)
    d_wq = nc.declare_dram_parameter("wq", [D, D], f32, isOutput=False)
    d_wk = nc.declare_dram_parameter("wk", [D, D], f32, isOutput=False)
    d_wv = nc.declare_dram_parameter("wv", [D, D], f32, isOutput=False)
    d_wp = nc.declare_dram_parameter("wp", [D, D], f32, isOutput=False)
    d_cq = nc.declare_dram_parameter("cq", [D, 1], f32, isOutput=False)
    d_ck = nc.declare_dram_parameter("ck", [D, 1], f32, isOutput=False)
    d_cv = nc.declare_dram_parameter("cv", [D, 1], f32, isOutput=False)
    d_id = nc.declare_dram_parameter("ident", [D, D], f32, isOutput=False)
    d_ones = nc.declare_dram_parameter("ones", [D, 32], f32, isOutput=False)
    d_out = nc.declare_dram_parameter("out", [LPC, 64, D], f32, isOutput=True)
    if debug:
        dbg = {nm: nc.declare_dram_parameter("dbg_" + nm, sh, f32, isOutput=True)
               for nm, sh in [("xnq", [128, QC, D]), ("qt", [128, TQ]), ("kt", [128, TK]),
                              ("vp", [KCS, KC, D]), ("pt0", [KCS, KC, TQ]),
                              ("at", [128, 384]), ("su", [128, 384]), ("zm", [128, 64])]}

    X = mybir.AxisListType.X
    SUB, MUL, ADD = mybir.AluOpType.subtract, mybir.AluOpType.mult, mybir.AluOpType.add

    from contextlib import ExitStack
    with tile.TileContext(nc) as tc, ExitStack() as es:
        cst = es.enter_context(tc.tile_pool(name="cst", bufs=1))
        xpool = es.enter_context(tc.tile_pool(name="x", bufs=1))
        st = es.enter_context(tc.tile_pool(name="st", bufs=1))
        sb = es.enter_context(tc.tile_pool(name="sb", bufs=2))
        ptp = es.enter_context(tc.tile_pool(name="pt", bufs=2))
        outp = es.enter_context(tc.tile_pool(name="outp", bufs=2))
        work = es.enter_context(tc.tile_pool(name="work", bufs=2, space="PSUM"))
        scp = es.enter_context(tc.tile_pool(name="scp", bufs=1, space="PSUM"))
        accp = es.enter_context(tc.tile_pool(name="accp", bufs=1, space="PSUM"))

        # constants
        wq_sb = cst.tile([D, D], f32, tag="wq"); nc.sync.dma_start(wq_sb[:], d_wq[:])
        wk_sb = cst.tile([D, D], f32, tag="wk"); nc.sync.dma_start(wk_sb[:], d_wk[:])
        wv_sb = cst.tile([D, D], f32, tag="wv"); nc.sync.dma_start(wv_sb[:], d_wv[:])
        wp_sb = cst.tile([D, D], f32, tag="wp"); nc.sync.dma_start(wp_sb[:], d_wp[:])
        id_sb = cst.tile([D, D], f32, tag="id"); nc.sync.dma_start(id_sb[:], d_id[:])
        on_sb = cst.tile([D, 32], f32, tag="on"); nc.sync.dma_start(on_sb[:], d_ones[:])
        cq_sb = cst.tile([D, 1], f32, tag="cq"); nc.sync.dma_start(cq_sb[:], d_cq[:])
        ck_sb = cst.tile([D, 1], f32, tag="ck"); nc.sync.dma_start(ck_sb[:], d_ck[:])
        cv_sb = cst.tile([D, 1], f32, tag="cv"); nc.sync.dma_start(cv_sb[:], d_cv[:])
        zc_sb = cst.tile([1, 512], f32, tag="zc"); nc.vector.memset(zc_sb[:], 0.0)

        # phase 1: load all x, bn_stats
        xq, xk, xv = [], [], []
        qs = st.tile([128, LPC * QC, 6], f32, tag="qs")
        ks = st.tile([KCS, LPC * KC * 2, 6], f32, tag="ks")
        for l in range(LPC):
            tq = xpool.tile([128, QC, D], f32, tag=f"xq{l}")
            nc.sync.dma_start(tq[:], d_q[l].rearrange("(c p) d -> p c d", p=128))
            xq.append(tq)
            tk = xpool.tile([KCS, KC, D], f32, tag=f"xk{l}")
            nc.sync.dma_start(tk[:], d_k[l].rearrange("(c p) d -> p c d", p=KCS))
            xk.append(tk)
            tv = xpool.tile([KCS, KC, D], f32, tag=f"xv{l}")
            nc.sync.dma_start(tv[:], d_v[l].rearrange("(c p) d -> p c d", p=KCS))
            xv.append(tv)
            for c in range(QC):
                nc.vector.bn_stats(qs[:, l * QC + c, :], tq[:, c, :])
            for c in range(KC):
                nc.vector.bn_stats(ks[:, l * KC * 2 + c, :], tk[:, c, :])
                nc.vector.bn_stats(ks[:, l * KC * 2 + KC + c, :], tv[:, c, :])

        # phase 2: combine even/odd stats -> mu, rstd   (chunk n even: ne=no=n/2)
        def stats_combine(s, P, NCH, n):
            me, mo = s[:P, :, 1:2].rearrange("p c o -> p (c o)"), s[:P, :, 4:5].rearrange("p c o -> p (c o)")
            m2e, m2o = s[:P, :, 2:3].rearrange("p c o -> p (c o)"), s[:P, :, 5:6].rearrange("p c o -> p (c o)")
            mu = st.tile([P, NCH], f32); rstd = st.tile([P, NCH], f32)
            tmp = st.tile([P, NCH], f32); m2 = st.tile([P, NCH], f32)
            nc.vector.tensor_tensor(out=tmp[:], in0=me, in1=mo, op=ADD)
            nc.vector.tensor_scalar_mul(mu[:], tmp[:], 0.5)
            nc.vector.tensor_tensor(out=tmp[:], in0=mo, in1=me, op=SUB)
            nc.vector.tensor_tensor(out=tmp[:], in0=tmp[:], in1=tmp[:], op=MUL)
            nc.vector.tensor_tensor(out=m2[:], in0=m2e, in1=m2o, op=ADD)
            nc.vector.scalar_tensor_tensor(out=m2[:], in0=tmp[:], scalar=float(n) / 4.0, in1=m2[:], op0=MUL, op1=ADD)
            # v = var + eps ; r0 = 1/sqrt_approx(v) ; one Newton step for the
            # loose ACT sqrt: r1 = r0*(1.5 - 0.5*v*r0^2)
            vv = st.tile([P, NCH], f32)
            nc.vector.tensor_scalar(out=vv[:], in0=m2[:], scalar1=1.0 / float(n),
                                    scalar2=float(LN_EPS), op0=MUL, op1=ADD)
            nc.scalar.activation(rstd[:], vv[:], mybir.ActivationFunctionType.Sqrt)
            nc.vector.reciprocal(rstd[:], rstd[:])
            t2 = st.tile([P, NCH], f32)
            nc.vector.tensor_tensor(out=t2[:], in0=rstd[:], in1=rstd[:], op=MUL)
            nc.vector.tensor_tensor(out=t2[:], in0=t2[:], in1=vv[:], op=MUL)
            nc.vector.tensor_scalar(out=t2[:], in0=t2[:], scalar1=-0.5, scalar2=1.5, op0=MUL, op1=ADD)
            nc.vector.tensor_tensor(out=rstd[:], in0=rstd[:], in1=t2[:], op=MUL)
            return mu, rstd
        qmu, qrstd = stats_combine(qs, 128, LPC * QC, 128)
        kmu, krstd = stats_combine(ks, KCS, LPC * KC * 2, D)

        # phase 3: per window-group pipeline
        for l in range(LPC):
            # LN apply (in place)
            for c in range(QC):
                i = l * QC + c
                nc.vector.tensor_scalar(out=xq[l][:, c, :], in0=xq[l][:, c, :],
                                        scalar1=qmu[:, i:i + 1], scalar2=qrstd[:, i:i + 1], op0=SUB, op1=MUL)
            for c in range(KC):
                i = l * KC * 2 + c
                nc.vector.tensor_scalar(out=xk[l][:, c, :], in0=xk[l][:, c, :],
                                        scalar1=kmu[:, i:i + 1], scalar2=krstd[:, i:i + 1], op0=SUB, op1=MUL)
                j = i + KC
                nc.vector.tensor_scalar(out=xv[l][:, c, :], in0=xv[l][:, c, :],
                                        scalar1=kmu[:, j:j + 1], scalar2=krstd[:, j:j + 1], op0=SUB, op1=MUL)

            # transpose xn -> [d, tokens] ; batched psum->sbuf copies
            tp_q = work.tile([128, 512], f32, tag="w")
            for c in range(QC):
                nc.tensor.transpose(tp_q[:, c * 128:(c + 1) * 128], xq[l][:, c, :], id_sb[:])
            xqT = sb.tile([128, TQ], f32, tag="xqT")
            nc.vector.tensor_copy(xqT[:], tp_q[:, 0:TQ])
            tp_k = work.tile([128, 512], f32, tag="w")
            for c in range(KC):
                nc.tensor.transpose(tp_k[:, c * KCS:(c + 1) * KCS], xk[l][:, c, :], id_sb[0:KCS, 0:KCS])
            xkT = sb.tile([128, TK], f32, tag="xkT")
            nc.vector.tensor_copy(xkT[:], tp_k[:, 0:TK])
            tp_v = work.tile([128, 512], f32, tag="w")
            for c in range(KC):
                nc.tensor.transpose(tp_v[:, c * KCS:(c + 1) * KCS], xv[l][:, c, :], id_sb[0:KCS, 0:KCS])
            xvT = sb.tile([128, TK], f32, tag="xvT")
            nc.vector.tensor_copy(xvT[:], tp_v[:, 0:TK])

            # projections
            if debug and l == 0:
                nc.sync.dma_start(dbg["xnq"][:], xq[l][:])
            qt_ps = work.tile([128, 512], f32, tag="w")
            nc.tensor.matmul(qt_ps[:, 0:TQ], wq_sb[:], xqT[:], start=True, stop=True)
            QT = sb.tile([128, TQ], f32, tag="QT")
            nc.vector.tensor_scalar(out=QT[:], in0=qt_ps[:, 0:TQ], scalar1=cq_sb[:, 0:1], scalar2=None, op0=ADD)
            kt_ps = work.tile([128, 512], f32, tag="w")
            nc.tensor.matmul(kt_ps[:, 0:TK], wk_sb[:], xkT[:], start=True, stop=True)
            KT = sb.tile([128, TK], f32, tag="KT")
            nc.vector.tensor_scalar(out=KT[:], in0=kt_ps[:, 0:TK], scalar1=ck_sb[:, 0:1], scalar2=None, op0=ADD)
            v_ps = work.tile([128, 512], f32, tag="w")
            for c in range(KC):
                nc.tensor.matmul(v_ps[0:KCS, c * 128:(c + 1) * 128].rearrange("p f -> p f"),
                                 xvT[:, c * KCS:(c + 1) * KCS], wv_sb[:], start=True, stop=True)
            Vp = sb.tile([KCS, KC, D], f32, tag="Vp")
            nc.vector.tensor_scalar(out=Vp[:].rearrange("p c d -> p (c d)"), in0=v_ps[0:KCS, 0:512],
                                    scalar1=cv_sb[0:KCS, 0:1], scalar2=None, op0=ADD)

            if debug and l == 0:
                nc.sync.dma_start(dbg["qt"][:], QT[:])
                nc.sync.dma_start(dbg["kt"][:], KT[:])
                nc.sync.dma_start(dbg["vp"][:], Vp[:])
            # attention
            at_ps = accp.tile([128, 384], f32, tag="at")
            su_ps = accp.tile([128, 384], f32, tag="su")
            # full-bank zeroing start matmuls: set has_written everywhere so the
            # 32 per-head matmuls below are pure order-independent accumulates
            nc.tensor.matmul(at_ps[:], zc_sb[0:1, 0:128], zc_sb[0:1, 0:384],
                             start=True, stop=False, skip_group_check=True)
            nc.tensor.matmul(su_ps[:], zc_sb[0:1, 0:128], zc_sb[0:1, 0:384],
                             start=True, stop=False, skip_group_check=True)
            nmm = 0
            for h in range(HEADS):
                hs = h * 32
                sc = scp.tile([128, KC, 512], f32, tag="sc")
                for c in range(KC):
                    nc.tensor.matmul(sc[0:KCS, c, 0:TQ], KT[hs:hs + 32, c * KCS:(c + 1) * KCS],
                                     QT[hs:hs + 32, :], start=True, stop=True, tile_position=(hs, 0))
                pt = ptp.tile([KCS, KC, TQ], f32, tag="pt")
                nc.scalar.activation(pt[:], sc[0:KCS, :, 0:TQ], mybir.ActivationFunctionType.Exp)
                if debug and l == 0 and h == 0:
                    nc.sync.dma_start(dbg["pt0"][:], pt[:])
                for c in range(KC):
                    nc.tensor.matmul(at_ps[hs:hs + 32, :], Vp[:, c, hs:hs + 32], pt[:, c, :],
                                     start=False, stop=(nmm == 2 * KC * HEADS - 2),
                                     tile_position=(0, hs), skip_group_check=True)
                    nc.tensor.matmul(su_ps[hs:hs + 32, :], on_sb[0:KCS, :], pt[:, c, :],
                                     start=False, stop=(nmm == 2 * KC * HEADS - 1),
                                     tile_position=(0, hs), skip_group_check=True)
                    nmm += 2

            R = sb.tile([128, 384], f32, tag="R")
            if debug and l == 0:
                su_dbg = sb.tile([128, 384], f32, tag="sudbg")
                nc.vector.tensor_copy(su_dbg[:], su_ps[:])
                nc.sync.dma_start(dbg["su"][:], su_dbg[:])
                at_dbg = sb.tile([128, 384], f32, tag="atdbg")
                nc.vector.tensor_copy(at_dbg[:], at_ps[:])
                nc.sync.dma_start(dbg["at"][:], at_dbg[:])
            nc.vector.reciprocal(R[:], su_ps[:])
            AT = sb.tile([128, 384], f32, tag="AT")
            nc.vector.tensor_tensor(out=AT[:], in0=at_ps[:], in1=R[:], op=MUL)

            zt_ps = work.tile([128, 512], f32, tag="w")
            nc.tensor.matmul(zt_ps[:, 0:TQ], wp_sb[:], AT[:], start=True, stop=True)
            zm = sb.tile([128, 64], f32, tag="zm")
            nc.vector.reduce_sum(zm[:], zt_ps[:, 0:TQ].rearrange("p (n w) -> p w n", n=6), axis=X)
            if debug and l == 0:
                nc.sync.dma_start(dbg["zm"][:], zm[:])
            zt2 = work.tile([128, 512], f32, tag="w")
            nc.tensor.transpose(zt2[0:64, 0:128], zm[:], id_sb[:])
            skip_sb = outp.tile([64, D], f32, tag="sk")
            nc.sync.dma_start(skip_sb[:], d_skip[l][:])
            o_sb = outp.tile([64, D], f32, tag="o")
            nc.vector.tensor_tensor(out=o_sb[:], in0=zt2[0:64, 0:128], in1=skip_sb[:], op=ADD)
            nc.sync.dma_start(d_out[l][:], o_sb[:])

    # -- walrus workaround #2: this build rejects >1 sync-wait per instruction;
    # hoist excess waits onto single-wait NOPs just before each instruction --
    nsplit = 0
    for fn in nc.m.functions:
        for blk in fn.blocks:
            insts = blk.instructions
            i = 0
            while i < len(insts):
                inst = insts[i]
                si = getattr(inst, "sync_info", None)
                waits = list(si.on_wait) if si is not None else []
                if len(waits) > 1:
                    inst.sync_info = mybir.SyncInfo(on_wait=waits[-1:], on_update=list(si.on_update))
                    for j, w in enumerate(waits[:-1]):
                        nop = mybir.InstNoOp(name=f"{inst.name}-sw{j}", ins=[], outs=[])
                        nop.engine = inst.engine
                        nop.sync_info = mybir.SyncInfo(on_wait=[w], on_update=[])
                        insts.insert(i, nop)
                        i += 1
                        nsplit += 1
                i += 1
    return nc


def _get_prog():
    if "nc" not in _prog:
        _prog["nc"] = _build_program()
    return _prog["nc"]


def kernel(q, k, v, skip, attn_scale, lnq_g, lnq_b, Wq, bq, lnk_g, lnk_b, Wk, bk,
           lnv_g, lnv_b, Wv, bv, Wp, bp):
    from concourse.bass_utils import run_bass_kernel_spmd

    q = np.asarray(q); k = np.asarray(k); v = np.asarray(v); skip = np.asarray(skip)
    b, n, Xd, Yd, W1, W2, d = q.shape
    _, _, x2, y2, w1, w2, _ = k.shape
    L = Xd * Yd
    # rearrange to [L, tokens, d] token-major (token = n*W1*W2 + w1*W2 + w2)
    qf = np.ascontiguousarray(q.transpose(0, 2, 3, 1, 4, 5, 6).reshape(L, n * W1 * W2, d))
    kf = np.ascontiguousarray(k.transpose(0, 2, 3, 1, 4, 5, 6).reshape(L, n * w1 * w2, d))
    vf = np.ascontiguousarray(v.transpose(0, 2, 3, 1, 4, 5, 6).reshape(L, n * w1 * w2, d))
    sf = np.ascontiguousarray(skip.reshape(L, W1 * W2, d) + bp[None, None, :]).astype(np.float32)

    s_tot = float(attn_scale.reshape(-1)[0]) * (DH ** -0.5)
    wq_eff = (lnq_g[:, None] * Wq * s_tot).astype(np.float32)
    cq = (lnq_b @ Wq * s_tot + bq * s_tot).astype(np.float32).reshape(D, 1)
    wk_eff = (lnk_g[:, None] * Wk).astype(np.float32)
    ck = (lnk_b @ Wk + bk).astype(np.float32).reshape(D, 1)
    wv_eff = (lnv_g[:, None] * Wv).astype(np.float32)
    cv = (lnv_b @ Wv + bv).astype(np.float32).reshape(D, 1)
    wp_eff = (Wp / float(n)).astype(np.float32)
    ident = np.eye(D, dtype=np.float32)
    ones = np.ones((D, 32), dtype=np.float32)

    nc = _get_prog()
    shared = dict(wq=wq_eff, wk=wk_eff, wv=wv_eff, wp=wp_eff, cq=cq, ck=ck, cv=cv,
                  ident=ident, ones=ones)
    in_maps = []
    for c in range(NCORES):
        s = slice(c * LPC, (c + 1) * LPC)
        in_maps.append(dict(q=np.ascontiguousarray(qf[s]), k=np.ascontiguousarray(kf[s]),
                            v=np.ascontiguousarray(vf[s]), skip=np.ascontiguousarray(sf[s]),
                            **shared))
    res = run_bass_kernel_spmd(nc, in_maps, core_ids=list(range(NCORES)))
    kernel._last_results = res
    outs = np.concatenate([res.results[c]["out"] for c in range(NCORES)], axis=0)  # [64, 64, 128]
    return outs.reshape(Xd, Yd, W1, W2, d)[None].astype(np.float32)



# revision 10
# speedup vs baseline: 1.7770x; 1.7770x over previous
"""CrossViewSwapAttention Trainium2 kernel (bf16 rewrite).

Sharding: 64 window groups L=X*Y split across 8 cores (8 groups/core); each
group's attention is local to a core.

Key optimizations over the fp32 baseline:
- All matmuls in bf16 (fp32 matmul runs at 1/4 PE rate).
- LayerNorm mean folded into the projection weights host-side (rank-1
  update), so the device only applies rstd scaling.
- K-side LN bias `ck` dropped entirely (adds a per-query constant to the
  logits -> softmax invariant). V-side LN bias `cv` folded into the output
  bias via cv @ Wp (softmax weights sum to 1). Q bias `cq` absorbed into the
  PSUM->SBUF cast of the Q projection. rstd_v applied in the V projection
  cast (per-partition scalar).
- Head-packed matmuls via tile_position so the 4 heads' score and AV
  matmuls run concurrently in disjoint 32-row/col PE bands.
- exp on ACT reads score PSUM directly; everything else is spread across
  DVE/GPSIMD to keep ACT exp-only.
"""
import numpy as np

HEADS, DH, D = 4, 32, 128
LN_EPS = 1e-5
NCORES = 8
LPC = 8          # L window-groups per core
TQ, TK = 384, 480
QC, QCS = 3, 128  # q chunks
KC, KCS = 4, 120  # kv chunks

_prog = {}


def _build_program():
    import concourse.bass as bass
    import concourse.tile as tile
    from concourse import mybir
    from concourse.tile import ScopedClock

    # -- walrus workaround: split tail-drain sem waits into single-wait NOPs --
    def _drain_and_barrier(self, tick_clock, wait_clock):
        nc = self.nc
        nop0 = nc.sync.nop()
        wait_clock.add_sem_waits(nop0.ins, ScopedClock({None: tick_clock.global_clock}))
        si = nop0.ins.sync_info
        waits = list(si.on_wait) if si is not None else []
        if len(waits) > 1:
            nop0.ins.sync_info = mybir.SyncInfo(on_wait=waits[:1], on_update=list(si.on_update))
            for w in waits[1:]:
                n = nc.sync.nop()
                n.ins.sync_info = mybir.SyncInfo(on_wait=[w], on_update=[])
        nc.sync.drain()
        nc.all_engine_barrier()
        assert self.sems is not None
        popped = nc._tile_sem_poison_stack.pop()
        assert popped is self._sem_poison
        nc.clear_and_free_semaphores(list(self.sems.allocated().values()))
        nc.all_engine_barrier()

    tile.TileContext._drain_and_barrier = _drain_and_barrier

    f32 = mybir.dt.float32
    bf16 = mybir.dt.bfloat16
    nc = bass.Bass()
    d_q = nc.declare_dram_parameter("q", [LPC, TQ, D], bf16, isOutput=False)
    d_k = nc.declare_dram_parameter("k", [LPC, TK, D], bf16, isOutput=False)
    d_v = nc.declare_dram_parameter("v", [LPC, TK, D], bf16, isOutput=False)
    d_skip = nc.declare_dram_parameter("skip", [LPC, 64, D], f32, isOutput=False)
    d_wq = nc.declare_dram_parameter("wq", [D, D], bf16, isOutput=False)
    d_wk = nc.declare_dram_parameter("wk", [D, D], bf16, isOutput=False)
    d_wv = nc.declare_dram_parameter("wv", [D, D], bf16, isOutput=False)
    d_wp = nc.declare_dram_parameter("wp", [D, D], bf16, isOutput=False)
    d_cq = nc.declare_dram_parameter("cq", [D, 1], f32, isOutput=False)
    d_id = nc.declare_dram_parameter("ident", [D, D], bf16, isOutput=False)
    d_id32 = nc.declare_dram_parameter("ident32", [D, D], f32, isOutput=False)
    d_ones = nc.declare_dram_parameter("ones", [KCS, 32], bf16, isOutput=False)
    d_out = nc.declare_dram_parameter("out", [LPC, 64, D], f32, isOutput=True)

    X = mybir.AxisListType.X
    SUB, MUL, ADD = mybir.AluOpType.subtract, mybir.AluOpType.mult, mybir.AluOpType.add

    from contextlib import ExitStack
    with tile.TileContext(nc) as tc, ExitStack() as es:
        cst = es.enter_context(tc.tile_pool(name="cst", bufs=1))
        xpool = es.enter_context(tc.tile_pool(name="x", bufs=1))
        st = es.enter_context(tc.tile_pool(name="st", bufs=1))
        sb = es.enter_context(tc.tile_pool(name="sb", bufs=2))
        ptp = es.enter_context(tc.tile_pool(name="pt", bufs=4))
        outp = es.enter_context(tc.tile_pool(name="outp", bufs=2))
        # PSUM: sc 2 banks x bufs=2 (4) + work 1 bank x bufs=2 (2) + at/su (2) = 8
        work = es.enter_context(tc.tile_pool(name="work", bufs=2, space="PSUM"))
        scp = es.enter_context(tc.tile_pool(name="scp", bufs=2, space="PSUM"))
        accp = es.enter_context(tc.tile_pool(name="accp", bufs=1, space="PSUM"))

        # constants
        wq_sb = cst.tile([D, D], bf16, tag="wq"); nc.sync.dma_start(wq_sb[:], d_wq[:])
        wk_sb = cst.tile([D, D], bf16, tag="wk"); nc.sync.dma_start(wk_sb[:], d_wk[:])
        wv_sb = cst.tile([D, D], bf16, tag="wv"); nc.sync.dma_start(wv_sb[:], d_wv[:])
        wp_sb = cst.tile([D, D], bf16, tag="wp"); nc.sync.dma_start(wp_sb[:], d_wp[:])
        id_sb = cst.tile([D, D], bf16, tag="id"); nc.sync.dma_start(id_sb[:], d_id[:])
        id32_sb = cst.tile([D, D], f32, tag="id32"); nc.sync.dma_start(id32_sb[:], d_id32[:])
        on_sb = cst.tile([KCS, 32], bf16, tag="on"); nc.sync.dma_start(on_sb[:], d_ones[:])
        cq_sb = cst.tile([D, 1], f32, tag="cq"); nc.sync.dma_start(cq_sb[:], d_cq[:])

        # phase 1: load all x, bn_stats (one instruction per tensor per group)
        xq, xk, xv = [], [], []
        qs = st.tile([128, LPC, QC, 6], f32, tag="qs")
        ks = st.tile([KCS, LPC, 2 * KC, 6], f32, tag="ks")
        for l in range(LPC):
            tq = xpool.tile([128, QC, D], bf16, tag=f"xq{l}")
            nc.sync.dma_start(tq[:], d_q[l].rearrange("(c p) d -> p c d", p=128))
            xq.append(tq)
            tk = xpool.tile([KCS, KC, D], bf16, tag=f"xk{l}")
            nc.sync.dma_start(tk[:], d_k[l].rearrange("(c p) d -> p c d", p=KCS))
            xk.append(tk)
            tv = xpool.tile([KCS, KC, D], bf16, tag=f"xv{l}")
            nc.sync.dma_start(tv[:], d_v[l].rearrange("(c p) d -> p c d", p=KCS))
            xv.append(tv)
            for c in range(QC):
                nc.vector.bn_stats(qs[:, l, c, :], tq[:, c, :])
            for c in range(KC):
                nc.vector.bn_stats(ks[:, l, c, :], tk[:, c, :])
                nc.vector.bn_stats(ks[:, l, KC + c, :], tv[:, c, :])

        # phase 2: combine even/odd stats -> rstd (chunk n even: ne=no=n/2)
        def stats_combine(s, P, NCH, n):
            me, mo = s[:P, :, 1:2].rearrange("p c o -> p (c o)"), s[:P, :, 4:5].rearrange("p c o -> p (c o)")
            m2e, m2o = s[:P, :, 2:3].rearrange("p c o -> p (c o)"), s[:P, :, 5:6].rearrange("p c o -> p (c o)")
            rstd = st.tile([P, NCH], f32)
            tmp = st.tile([P, NCH], f32); m2 = st.tile([P, NCH], f32)
            nc.vector.tensor_tensor(out=tmp[:], in0=mo, in1=me, op=SUB)
            nc.vector.tensor_tensor(out=tmp[:], in0=tmp[:], in1=tmp[:], op=MUL)
            nc.vector.tensor_tensor(out=m2[:], in0=m2e, in1=m2o, op=ADD)
            nc.vector.scalar_tensor_tensor(out=m2[:], in0=tmp[:], scalar=float(n) / 4.0, in1=m2[:], op0=MUL, op1=ADD)
            # v = var + eps ; r0 = 1/sqrt_approx(v) ; one Newton step for the
            # loose ACT sqrt: r1 = r0*(1.5 - 0.5*v*r0^2)
            vv = st.tile([P, NCH], f32)
            nc.vector.tensor_scalar(out=vv[:], in0=m2[:], scalar1=1.0 / float(n),
                                    scalar2=float(LN_EPS), op0=MUL, op1=ADD)
            nc.scalar.activation(rstd[:], vv[:], mybir.ActivationFunctionType.Sqrt)
            nc.vector.reciprocal(rstd[:], rstd[:])
            t2 = st.tile([P, NCH], f32)
            nc.vector.tensor_tensor(out=t2[:], in0=rstd[:], in1=rstd[:], op=MUL)
            nc.vector.tensor_tensor(out=t2[:], in0=t2[:], in1=vv[:], op=MUL)
            nc.vector.tensor_scalar(out=t2[:], in0=t2[:], scalar1=-0.5, scalar2=1.5, op0=MUL, op1=ADD)
            nc.vector.tensor_tensor(out=rstd[:], in0=rstd[:], in1=t2[:], op=MUL)
            return rstd
        qrstd = stats_combine(qs.rearrange("p l c o -> p (l c) o"), 128, LPC * QC, 128)
        krstd = stats_combine(ks.rearrange("p l c o -> p (l c) o"), KCS, LPC * KC * 2, D)

        # phase 3: per window-group pipeline
        for l in range(LPC):
            # rstd apply (in place, bf16 2x) -- mean is folded into W host-side
            for c in range(QC):
                i = l * QC + c
                nc.gpsimd.tensor_scalar(out=xq[l][:, c, :], in0=xq[l][:, c, :],
                                        scalar1=qrstd[:, i:i + 1], scalar2=None, op0=MUL)
            for c in range(KC):
                i = l * KC * 2 + c
                nc.gpsimd.tensor_scalar(out=xk[l][:, c, :], in0=xk[l][:, c, :],
                                        scalar1=krstd[:, i:i + 1], scalar2=None, op0=MUL)
            # (v rstd is applied at the Vp cast, per-partition there)

            # transpose x -> [d, tokens]
            tp_q = work.tile([128, 1024], bf16, tag="w")
            for c in range(QC):
                nc.tensor.transpose(tp_q[:, c * 128:(c + 1) * 128], xq[l][:, c, :], id_sb[:])
            xqT = sb.tile([128, TQ], bf16, tag="xqT")
            nc.vector.tensor_copy(xqT[:], tp_q[:, 0:TQ])
            tp_k = work.tile([128, 1024], bf16, tag="w")
            for c in range(KC):
                nc.tensor.transpose(tp_k[:, c * KCS:(c + 1) * KCS], xk[l][:, c, :], id_sb[0:KCS, 0:KCS])
            xkT = sb.tile([128, TK], bf16, tag="xkT")
            nc.vector.tensor_copy(xkT[:], tp_k[:, 0:TK])
            tp_v = work.tile([128, 1024], bf16, tag="w")
            for c in range(KC):
                nc.tensor.transpose(tp_v[:, c * KCS:(c + 1) * KCS], xv[l][:, c, :], id_sb[0:KCS, 0:KCS])
            xvT = sb.tile([128, TK], bf16, tag="xvT")
            nc.vector.tensor_copy(xvT[:], tp_v[:, 0:TK])

            # projections
            qt_ps = work.tile([128, 512], f32, tag="w")
            nc.tensor.matmul(qt_ps[:, 0:TQ], wq_sb[:], xqT[:], start=True, stop=True)
            QT = sb.tile([128, TQ], bf16, tag="QT")
            nc.vector.tensor_scalar(out=QT[:], in0=qt_ps[:, 0:TQ], scalar1=cq_sb[:, 0:1], scalar2=None, op0=ADD)
            kt_ps = work.tile([128, 512], f32, tag="w")
            nc.tensor.matmul(kt_ps[:, 0:TK], wk_sb[:], xkT[:], start=True, stop=True)
            KT = sb.tile([128, TK], bf16, tag="KT")
            nc.vector.tensor_copy(KT[:], kt_ps[:, 0:TK])
            v_ps = work.tile([128, 512], f32, tag="w")
            for c in range(KC):
                nc.tensor.matmul(v_ps[0:KCS, c * 128:(c + 1) * 128],
                                 xvT[:, c * KCS:(c + 1) * KCS], wv_sb[:], start=True, stop=True)
            Vp = sb.tile([KCS, KC, D], bf16, tag="Vp")
            for c in range(KC):
                i = l * KC * 2 + KC + c
                nc.vector.tensor_scalar(out=Vp[:, c, :], in0=v_ps[0:KCS, c * 128:(c + 1) * 128],
                                        scalar1=krstd[:, i:i + 1], scalar2=None, op0=MUL)

            # attention: per (head, kv-half): scores -> exp -> at/su. The at/su
            # matmuls of (h, half) run on the PE inside the ACT window of the
            # next half's exp, and bands are independent via tile_position.
            at_ps = accp.tile([128, 384], f32, tag="at")
            su_ps = accp.tile([128, 384], f32, tag="su")
            for h in range(HEADS):
                hs = h * 32
                for half in range(2):
                    sc = scp.tile([KCS, 2, 512], f32, tag="sc")
                    for ci in range(2):
                        c = half * 2 + ci
                        nc.tensor.matmul(sc[0:KCS, ci, 0:TQ], KT[hs:hs + 32, c * KCS:(c + 1) * KCS],
                                         QT[hs:hs + 32, :], start=True, stop=True, tile_position=(hs, 0))
                    pt = ptp.tile([KCS, 2, TQ], bf16, tag="pt")
                    nc.scalar.activation(pt[:], sc[0:KCS, :, 0:TQ], mybir.ActivationFunctionType.Exp)
                    last = half == 1
                    for ci in range(2):
                        first = half == 0 and ci == 0
                        nc.tensor.matmul(at_ps[hs:hs + 32, :], Vp[:, half * 2 + ci, hs:hs + 32], pt[:, ci, :],
                                         start=first, stop=(last and ci == 1),
                                         tile_position=(0, hs), skip_group_check=True)
                        nc.tensor.matmul(su_ps[hs:hs + 32, :], on_sb[:], pt[:, ci, :],
                                         start=first, stop=(last and ci == 1),
                                         tile_position=(0, hs), skip_group_check=True)

            R = sb.tile([128, 384], f32, tag="R")
            nc.vector.reciprocal(R[:], su_ps[:])
            AT = sb.tile([128, 384], bf16, tag="AT")
            nc.vector.tensor_tensor(out=AT[:], in0=at_ps[:], in1=R[:], op=MUL)

            zt_ps = work.tile([128, 512], f32, tag="w")
            nc.tensor.matmul(zt_ps[:, 0:TQ], wp_sb[:], AT[:], start=True, stop=True)
            zm = sb.tile([128, 64], f32, tag="zm")
            nc.vector.reduce_sum(zm[:], zt_ps[:, 0:TQ].rearrange("p (n w) -> p w n", n=6), axis=X)
            zt2 = work.tile([128, 512], f32, tag="w")
            nc.tensor.transpose(zt2[0:64, 0:128], zm[:], id32_sb[:])
            skip_sb = outp.tile([64, D], f32, tag="sk")
            nc.sync.dma_start(skip_sb[:], d_skip[l][:])
            o_sb = outp.tile([64, D], f32, tag="o")
            nc.vector.tensor_tensor(out=o_sb[:], in0=zt2[0:64, 0:128], in1=skip_sb[:], op=ADD)
            nc.sync.dma_start(d_out[l][:], o_sb[:])

    # -- walrus workaround #2: this build rejects >1 sync-wait per instruction;
    # hoist excess waits onto single-wait NOPs just before each instruction --
    for fn in nc.m.functions:
        for blk in fn.blocks:
            insts = blk.instructions
            i = 0
            while i < len(insts):
                inst = insts[i]
                si = getattr(inst, "sync_info", None)
                waits = list(si.on_wait) if si is not None else []
                if len(waits) > 1:
                    inst.sync_info = mybir.SyncInfo(on_wait=waits[-1:], on_update=list(si.on_update))
                    for j, w in enumerate(waits[:-1]):
                        nop = mybir.InstNoOp(name=f"{inst.name}-sw{j}", ins=[], outs=[])
                        nop.engine = inst.engine
                        nop.sync_info = mybir.SyncInfo(on_wait=[w], on_update=[])
                        insts.insert(i, nop)
                        i += 1
                i += 1
    return nc


def _get_prog():
    if "nc" not in _prog:
        _prog["nc"] = _build_program()
    return _prog["nc"]


def kernel(q, k, v, skip, attn_scale, lnq_g, lnq_b, Wq, bq, lnk_g, lnk_b, Wk, bk,
           lnv_g, lnv_b, Wv, bv, Wp, bp):
    import ml_dtypes
    from concourse.bass_utils import run_bass_kernel_spmd

    bf16 = ml_dtypes.bfloat16
    q = np.asarray(q); k = np.asarray(k); v = np.asarray(v); skip = np.asarray(skip)
    b, n, Xd, Yd, W1, W2, d = q.shape
    _, _, x2, y2, w1, w2, _ = k.shape
    L = Xd * Yd
    # rearrange to [L, tokens, d] token-major (token = n*W1*W2 + w1*W2 + w2)
    qf = np.ascontiguousarray(q.transpose(0, 2, 3, 1, 4, 5, 6).reshape(L, n * W1 * W2, d)).astype(bf16)
    kf = np.ascontiguousarray(k.transpose(0, 2, 3, 1, 4, 5, 6).reshape(L, n * w1 * w2, d)).astype(bf16)
    vf = np.ascontiguousarray(v.transpose(0, 2, 3, 1, 4, 5, 6).reshape(L, n * w1 * w2, d)).astype(bf16)

    s_tot = float(attn_scale.reshape(-1)[0]) * (DH ** -0.5)

    def mean_fold(W):
        return W - np.mean(W, axis=0, keepdims=True)

    wq_eff = mean_fold(lnq_g[:, None] * Wq * s_tot).astype(bf16)
    cq = (lnq_b @ Wq * s_tot + bq * s_tot).astype(np.float32).reshape(D, 1)
    wk_eff = mean_fold(lnk_g[:, None] * Wk).astype(bf16)
    # ck = lnk_b @ Wk + bk adds a per-query constant to every logit -> softmax
    # invariant -> dropped.
    wv_eff = mean_fold(lnv_g[:, None] * Wv).astype(bf16)
    cv = (lnv_b @ Wv + bv).astype(np.float32)
    wp_eff = (Wp / float(n)).astype(bf16)
    # skip + bp + cv @ Wp (cv passes through softmax-normalized attention as a
    # constant, then through the projection; mean over views keeps it as-is)
    sf = np.ascontiguousarray(
        skip.reshape(L, W1 * W2, d) + bp[None, None, :] + (cv @ Wp)[None, None, :]
    ).astype(np.float32)
    ident = np.eye(D, dtype=bf16)
    ident32 = np.eye(D, dtype=np.float32)
    ones = np.ones((KCS, 32), dtype=bf16)

    nc = _get_prog()
    shared = dict(wq=wq_eff, wk=wk_eff, wv=wv_eff, wp=wp_eff, cq=cq,
                  ident=ident, ident32=ident32, ones=ones)
    in_maps = []
    for c in range(NCORES):
        s = slice(c * LPC, (c + 1) * LPC)
        in_maps.append(dict(q=np.ascontiguousarray(qf[s]), k=np.ascontiguousarray(kf[s]),
                            v=np.ascontiguousarray(vf[s]), skip=np.ascontiguousarray(sf[s]),
                            **shared))
    res = run_bass_kernel_spmd(nc, in_maps, core_ids=list(range(NCORES)))
    kernel._last_results = res
    outs = np.concatenate([res.results[c]["out"] for c in range(NCORES)], axis=0)  # [64, 64, 128]
    return outs.reshape(Xd, Yd, W1, W2, d)[None].astype(np.float32)
